# revision 50
# baseline (speedup 1.0000x reference)
"""Trainium2 Bass kernel for nn_CLsLoss (ABCD soft-region weighted histograms +
profile likelihood).

Strategy (data-parallel over events, 8 cores):
  - Each core gets 1/8 of the 4M bkg events and 1/8 of the 4M sig events,
    reshaped to [128, COLS] (zero-padded weights for the tail). bkg and sig
    chunks are interleaved host-side so each on-device chunk of J columns
    holds J/2 bkg columns followed by J/2 sig columns and every elementwise
    op covers both datasets in one instruction.
  - Per event on-device: sigmoids s1,s2 (ScalarE, bias APs carry the runtime
    cuts); bin index idx = floor((mt-e0)/w) via one ScalarE affine with the
    HW round-to-nearest int16 conversion and a -0.5 bias.
  - Cumulative step slabs instead of one-hots: G_m = [idx >= 2m] for
    m = 0..24; the host recovers the pair histogram as G[m] - G[m+1]
    (G[25] = 0 identically since idx <= 49, so slab 25 is never computed).
    Engine split per the cost model's LP optimum: slab 0 (all ones) is a
    GpSimd memset, ~4 slabs ride ScalarE as saturated sigmoids (same table
    set as the region sigmoids), ~1 on GpSimd, the rest on VectorE is_ge
    (4x mode).
  - Payload: 8 slabs [d | d&mask] where d = (w, w*s1, w*s2, w*s1*s2) in
    bf16. The odd-half product d*(idx&1) is ONE GpSimd tensor_tensor
    bitwise_and over all 4 channels against the broadcast sign-mask
    m16 = -(idx&1) in {0x0000, 0xFFFF} (bit-exact multiply by 0/1, and
    GpSimd's launch overhead amortizes over the 4J-wide fused op, which is
    where GpSimd has comparative advantage per the cost model).
  - Histogram via TensorE: per column t,
      psum[25, 8] += G[128, 25]^T @ SD[128, 8]
    accumulated over all columns per dataset in one PSUM fp32 group.
  - Host: sum per-core [25, 16] partials, difference the steps, map
    (m, odd) -> bin, derive regions A=H1-H12, B=H12, C=H-H1-H2+H12,
    D=H2-H12, scale by INT_LUMI, and evaluate the [50]-bin profile
    likelihood in float64.
"""

import os as _os

import numpy as np

NBIN = 50
N_EVENTS = 4_000_000
NCORES = 8
NPC = N_EVENTS // NCORES          # 500_000 events per core per dataset
P = 128
COLS = int(_os.environ.get("K_COLS", "3920"))   # 128*3920 >= NPC, tail w=0
JMAX = int(_os.environ.get("K_J", "1056"))      # joint chunk width (both ds)
Q = 25                            # pair index m = idx >> 1 (slab 25 == 0)
NCH = 4                           # channels: w, w*s1, w*s2, w*s1*s2
NSLAB = 2 * NCH                   # payload slabs: [d | d*s]
PACK = 4                          # columns per matmul
QOH_GP = int(_os.environ.get("K_QOH_GP", "5"))    # step slabs on GpSimd
QOH_ACT = int(_os.environ.get("K_QOH_ACT", "4"))  # step slabs on ScalarE
QOH_BUFS = int(_os.environ.get("K_QOH_BUFS", "2"))
TAILSHIFT = int(_os.environ.get("K_TAILSHIFT", "2"))  # tail-chunk DVE->ACT steps
HEADSHIFT = int(_os.environ.get("K_HEADSHIFT", "0"))  # head-chunk ACT->DVE steps
# fractional DVE->ACT balance: split one step slab, DVE cols [0:J*num/den),
# ScalarE the rest (the whole-slab quantum overshoots ScalarE's slack)
SPLIT = int(_os.environ.get("K_SPLIT", "1"))          # 1 = enable split slab
SPLIT_NUM = int(_os.environ.get("K_SPLIT_NUM", "4"))
SPLIT_DEN = int(_os.environ.get("K_SPLIT_DEN", "16"))
SPLIT_POOL = int(_os.environ.get("K_SPLIT_POOL", "5"))  # GpSimd's share/den
# ones-slab engine: GpSimd memset (975 ns) vs VectorE is_ge-vs-(-1) (335 ns,
# 4x TSP) - the latter costs 640 ns/chunk less total work
ONES_DVE = int(_os.environ.get("K_ONES_DVE", "0"))
# optional second split slab (same DEN); 0 disables
SPLIT2_NUM = int(_os.environ.get("K_SPLIT2_NUM", "0"))
SPLIT2_POOL = int(_os.environ.get("K_SPLIT2_POOL", "0"))
STEP_SLOPE = 40.0  # sigmoid(+-20) rounds to exactly 1/0 in bf16
INT_LUMI = 117100.0
EPS = 1e-6
STEEPNESS = 20.0


def _chunks():
    """Split per-dataset COLS into chunk widths of at most JMAX//2, each a
    multiple of PACK. The first chunks are small so the pipeline fills
    quickly (less serial head time); the last chunks are small so the final
    matmul burst + drain ceremony hangs off a tiny dependency tail."""
    half = JMAX // 2
    head = [int(x) for x in
            _os.environ.get("K_HEAD", "84,288").split(",") if x]
    tail = [int(x) for x in
            _os.environ.get("K_TAIL", "64").split(",") if x]
    mid = COLS - sum(head) - sum(tail)
    assert mid > 0 and mid % PACK == 0
    widths = list(head)
    while mid > half:
        widths.append(half)
        mid -= half
    if mid:
        widths.append(mid)
    widths += tail
    # chunk = (c0b, wb, c0s, ws): per-dataset start/width. Normally
    # symmetric; with K_ASYM the final chunk is sig-only (its bkg share
    # folds into the penultimate chunk) so the bkg PSUM copy + output-DMA
    # launch ceremony overlaps the last chunk's compute instead of
    # serializing after it.
    wb_list = list(widths)
    ws_list = list(widths)
    if int(_os.environ.get("K_ASYM", "0")) and len(widths) >= 2:
        wb_list[-2] += wb_list[-1]
        wb_list[-1] = 0
    out = []
    c0b = c0s = 0
    for wb, ws in zip(wb_list, ws_list):
        assert wb % PACK == 0 and ws % PACK == 0 and wb + ws > 0
        out.append((c0b, wb, c0s, ws))
        c0b += wb
        c0s += ws
    assert c0b == COLS and c0s == COLS
    return out


def _build_program():
    import concourse.bass as bass
    import concourse.bacc as bacc
    import concourse.mybir as mybir
    import concourse.tile as tile

    dt = mybir.dt
    Alu = mybir.AluOpType
    Act = mybir.ActivationFunctionType

    nc = bacc.Bacc("TRN2", target_bir_lowering=False, debug=False,
                   num_devices=NCORES)

    # Joint inputs: [:, 0:COLS] = bkg, [:, COLS:2*COLS] = sig, chunk-
    # interleaved by the host so one chunk slice is contiguous.
    names = ["f1", "f2", "mt", "w"]
    din = {n: nc.dram_tensor(n, [P, 2 * COLS], dt.float32,
                             kind="ExternalInput")
           for n in names}
    dpar = nc.dram_tensor("params", [P, 16], dt.float32, kind="ExternalInput")
    dout = nc.dram_tensor("hist_out", [Q, 2 * NSLAB], dt.float32,
                          kind="ExternalOutput")

    chunks = _chunks()

    from contextlib import ExitStack
    with tile.TileContext(nc) as tc, ExitStack() as ctx:
        io_pool = ctx.enter_context(tc.tile_pool(name="io", bufs=2))
        act_pool = ctx.enter_context(tc.tile_pool(
            name="acto", bufs=int(_os.environ.get("K_ACT_BUFS", "2"))))
        sd_pool = ctx.enter_context(tc.tile_pool(name="sd", bufs=2))
        qoh_pool = ctx.enter_context(tc.tile_pool(name="qoh", bufs=QOH_BUFS))
        const_pool = ctx.enter_context(tc.tile_pool(name="const", bufs=1))
        psum_pool = ctx.enter_context(
            tc.tile_pool(name="psum", bufs=1, space=bass.MemorySpace.PSUM))
        out_pool = ctx.enter_context(tc.tile_pool(name="out", bufs=1))

        par = const_pool.tile([P, 16], dt.float32)
        warm = const_pool.tile([P, 16], dt.bfloat16)
        warm_src = const_pool.tile([P, 16], dt.bfloat16)
        # memset has no input deps, so the sigmoid table load starts at t=0
        # and fully overlaps the params + first-chunk DMAs
        nc.gpsimd.memset(warm_src[:], 0.0)
        # params ride the ACT DGE ring so the first mt chunk is not queued
        # behind them on the SP ring
        nc.scalar.dma_start(par[:], dpar[:])
        bias1 = par[:, 0:1]     # -20*cut1
        bias2 = par[:, 1:2]     # -20*cut2
        invw = par[:, 2:3]      # 1/bin_width
        nege0h = par[:, 3:4]    # -edges[0]/bin_width - 0.5  (floor via rint)
        # touch Sigmoid early so the ACT table set loads during input DMA
        nc.scalar.activation(warm[:], warm_src[:], Act.Sigmoid)

        ps = {ds: psum_pool.tile([Q, NSLAB], dt.float32,
                                 name=f"ps_{ds}", tag=f"ps_{ds}")
              for ds in ("bkg", "sig")}
        started = {"bkg": False, "sig": False}
        n_packs = {"bkg": COLS, "sig": COLS}
        done_packs = {"bkg": 0, "sig": 0}
        drained = set()
        out_sb = out_pool.tile([Q, 2 * NSLAB], dt.float32)

        n_base = Q - QOH_GP - QOH_ACT  # ACT slabs sit below GpSimd's

        for ci, (c0b, wb, c0s, ws) in enumerate(chunks):
            J = wb + ws  # joint width: wb bkg cols then ws sig cols
            off = c0b + c0s  # running dram column offset of this chunk
            w = wb  # bkg/sig boundary within the chunk
            f1 = io_pool.tile([P, J], dt.float32, tag="f1")
            f2 = io_pool.tile([P, J], dt.float32, tag="f2")
            mt = io_pool.tile([P, J], dt.float32, tag="mt")
            wt = io_pool.tile([P, J], dt.float32, tag="w")
            # host interleaves so joint chunk k occupies cols [off, off+J);
            # mt first: it feeds idx16 -> m16 -> all step slabs
            nc.sync.dma_start(mt[:], din["mt"][:, off:off + J])
            nc.sync.dma_start(f1[:], din["f1"][:, off:off + J])
            nc.sync.dma_start(f2[:], din["f2"][:, off:off + J])
            nc.sync.dma_start(wt[:], din["w"][:, off:off + J])

            s12 = act_pool.tile([P, 2 * J], dt.bfloat16, tag="s12")
            s1 = s12[:, 0:J]
            s2 = s12[:, J:2 * J]
            idx16 = act_pool.tile([P, J], dt.int16, tag="idx16")
            # parity mask {0, 1}: min-multiplier for the odd-half payload
            # (payloads are <= w_max ~ 1e-3 < 1, so min(d, parity) == d*parity)
            m16 = act_pool.tile([P, J], dt.int16, tag="m16")
            sd = sd_pool.tile([P, NSLAB * J], dt.bfloat16, tag="sd")

            # idx = floor((mt - e0)/binw) via rint(x - 0.5) on ACT -> int16;
            # first so m16 and the slab engines start as early as possible
            nc.scalar.activation(idx16[:], mt[:], Act.Identity,
                                 bias=nege0h, scale=invw)
            nc.scalar.activation(s1, f1[:], Act.Sigmoid,
                                 bias=bias1, scale=STEEPNESS)
            nc.scalar.activation(s2, f2[:], Act.Sigmoid,
                                 bias=bias2, scale=STEEPNESS)
            # d slab 0 = w in bf16 (ACT copy straight into the payload
            # tile; Identity, not Copy, so no extra act-table load)
            nc.scalar.activation(sd[:, 0:J], wt[:], Act.Identity)

            # small head/tail chunks skip GpSimd for payload+steps: its
            # per-op launch overhead dominates there, and the tail chunks
            # must clear elementwise ASAP so the final matmuls start early
            small = w < int(_os.environ.get("K_SMALL", "0"))

            # m16 = idx & 1 (odd/even split of each bin pair)
            nc.vector.tensor_scalar(m16[:], idx16[:], 1, None,
                                    Alu.bitwise_and)
            # d channels
            # (d1|d2) = w*(s1|s2) in one broadcast TT; d3 = d1*s2 on GpSimd
            # (a mult rides GpSimd's Multiply ucode; the cost model's LP puts
            # d3 + half the odd product there to balance the three engines)
            w_b = sd[:, 0:J].rearrange("p (o t) -> p o t", o=1)
            w_b = w_b.to_broadcast((P, 2, J))
            nc.vector.tensor_tensor(
                sd[:, J:3 * J].rearrange("p (r t) -> p r t", r=2),
                w_b, s12[:].rearrange("p (r t) -> p r t", r=2), Alu.mult)
            nc.vector.tensor_tensor(sd[:, 3 * J:4 * J], sd[:, J:2 * J],
                                    s2, Alu.mult)
            # odd half: d * (idx&1), one broadcast TT. All payload stays on
            # VectorE: GpSimd then depends only on idx16 (pure step slabs),
            # which keeps the cross-engine dependency chains short and the
            # pipeline stall-free
            d_all = sd[:, 0:4 * J].rearrange("p (r t) -> p r t", r=4)
            m_b = m16[:].rearrange("p (o t) -> p o t", o=1)
            nc.vector.tensor_tensor(
                sd[:, 4 * J:8 * J].rearrange("p (r t) -> p r t", r=4),
                m_b.to_broadcast((P, 4, J)), d_all, Alu.mult)

            # cumulative step slabs: G_m[i] = [idx_i >= 2m]. The host
            # recovers pair histograms as G[m] - G[m+1] (G[25] == 0). Slab 0
            # ([idx >= 0] == 1) is a GpSimd memset; QOH_ACT slabs ride
            # ScalarE as saturated sigmoids (same table set as s1/s2);
            # QOH_GP on GpSimd; the rest on VectorE (4x is_ge).
            qoh = qoh_pool.tile([P, Q * J], dt.bfloat16, tag="qoh")
            # tail chunks: shift step slabs off VectorE (it is the engine
            # that finishes last) onto ScalarE, so the final matmul burst
            # starts earlier
            tshift = TAILSHIFT if ci >= len(chunks) - 2 else 0
            # head chunks: ScalarE gates the whole cascade (idx/sigmoids),
            # so move its step slabs to VectorE there
            hshift = HEADSHIFT if ci < 2 else 0
            act_set = set(range(n_base - tshift + hshift, n_base + QOH_ACT))
            # split slab: the one just below the ScalarE range, steady
            # chunks only
            m_split = (n_base - TAILSHIFT - 1
                       if SPLIT and tshift == 0 and hshift == 0 else -1)
            m_split2 = (n_base - TAILSHIFT - 2
                        if SPLIT2_NUM and m_split >= 0 else -1)
            pool_set = (set() if small
                        else set(range(Q - QOH_GP, Q)) - act_set)
            for m in range(Q):
                slab = qoh[:, m * J:(m + 1) * J]
                if m == 0:
                    if ONES_DVE:
                        nc.vector.tensor_scalar(slab, idx16[:], -1.0,
                                                None, Alu.is_ge)
                    else:
                        nc.gpsimd.memset(slab, 1.0)
                elif m in (m_split, m_split2):
                    num, pnum = ((SPLIT_NUM, SPLIT_POOL) if m == m_split
                                 else (SPLIT2_NUM, SPLIT2_POOL))
                    Jd = (J * num // SPLIT_DEN) & ~1
                    Jp = (J * pnum // SPLIT_DEN) & ~1
                    i = m - (n_base - TAILSHIFT - 2)
                    negm = par[:, 4 + i:5 + i]  # -SLOPE*(2m - 0.5)
                    nc.vector.tensor_scalar(slab[:, 0:Jd], idx16[:, 0:Jd],
                                            float(2 * m), None, Alu.is_ge)
                    if Jp:
                        nc.gpsimd.tensor_scalar(
                            slab[:, Jd:Jd + Jp], idx16[:, Jd:Jd + Jp],
                            float(2 * m), None, Alu.is_ge)
                    nc.scalar.activation(slab[:, Jd + Jp:],
                                         idx16[:, Jd + Jp:],
                                         Act.Sigmoid, bias=negm,
                                         scale=STEP_SLOPE)
                elif m in act_set:
                    i = m - (n_base - TAILSHIFT - 2)
                    negm = par[:, 4 + i:5 + i]  # -SLOPE*(2m - 0.5)
                    nc.scalar.activation(slab, idx16[:], Act.Sigmoid,
                                         bias=negm, scale=STEP_SLOPE)
                elif m in pool_set:
                    nc.gpsimd.tensor_scalar(slab, idx16[:], float(2 * m),
                                            None, Alu.is_ge)
                else:
                    nc.vector.tensor_scalar(slab, idx16[:], float(2 * m),
                                            None, Alu.is_ge)

            # per-column matmuls (hw: the weights AP allows only one free
            # dim, so columns cannot be packed into a wider stationary)
            qoh_r = qoh[:].rearrange("p (m t) -> p t m", t=J)
            sd_r = sd[:].rearrange("p (j t) -> p t j", t=J)
            for t0 in range(J):
                ds = "bkg" if t0 < w else "sig"
                first = not started[ds]
                started[ds] = True
                done_packs[ds] += 1
                last = done_packs[ds] == n_packs[ds]
                nc.tensor.matmul(
                    ps[ds][:], qoh_r[:, t0, :], sd_r[:, t0, :],
                    start=first, stop=last, skip_group_check=True)

            # with the asymmetric tail, bkg completes one chunk early: its
            # PSUM copy + output-DMA launch ceremony (~1.5us of sem/DGE
            # fixed cost) overlaps the final chunk's compute instead of
            # serializing after it. ScalarE does the early copy - it is
            # idle at the tail while VectorE is the critical engine.
            if (done_packs["bkg"] == n_packs["bkg"] and "bkg" not in drained
                    and ci < len(chunks) - 1):
                drained.add("bkg")
                nc.scalar.activation(out_sb[:, 0:NSLAB], ps["bkg"][:],
                                     Act.Identity)
                nc.sync.dma_start(dout[:, 0:NSLAB], out_sb[:, 0:NSLAB])

        if "bkg" in drained:
            nc.vector.tensor_copy(out_sb[:, NSLAB:], ps["sig"][:])
            nc.sync.dma_start(dout[:, NSLAB:], out_sb[:, NSLAB:])
        else:
            nc.vector.tensor_copy(out_sb[:, 0:NSLAB], ps["bkg"][:])
            nc.vector.tensor_copy(out_sb[:, NSLAB:], ps["sig"][:])
            nc.sync.dma_start(dout[:], out_sb[:])

    nc.compile()
    return nc


def _shard_joint(arr: np.ndarray, core: int, chunks) -> np.ndarray:
    """arr: (bkg_full, sig_full) pair -> [P, 2*COLS] chunk-interleaved."""
    bkg_full, sig_full = arr
    out = np.zeros((P, 2 * COLS), dtype=np.float32)
    halves = []
    for full in (bkg_full, sig_full):
        sl = full[core * NPC:(core + 1) * NPC]
        h = np.zeros(P * COLS, dtype=np.float32)
        h[:NPC] = sl
        halves.append(h.reshape(P, COLS))
    b, s = halves
    for c0b, wb, c0s, ws in chunks:
        off = c0b + c0s
        out[:, off:off + wb] = b[:, c0b:c0b + wb]
        out[:, off + wb:off + wb + ws] = s[:, c0s:c0s + ws]
    return out


def _decode(block: np.ndarray) -> np.ndarray:
    """[Q, NSLAB] psum block of cumulative steps -> [NBIN, NCH] histogram.

    Row m holds G[m] = sum over events with idx >= 2m; pair m is
    G[m] - G[m+1] (G[25] = 0 since idx <= 49)."""
    pair = block.astype(np.float64).copy()
    pair[:-1] -= block[1:]
    h_all = pair[:, 0:NCH]
    h_odd = pair[:, NCH:2 * NCH]
    h_even = h_all - h_odd
    bins = np.empty((2 * Q, NCH))
    bins[0::2] = h_even
    bins[1::2] = h_odd
    return bins[:NBIN]


def _regions(h: np.ndarray) -> np.ndarray:
    """[NBIN, 4] channel hist (H, H1, H2, H12) -> regions (A,B,C,D)*lumi."""
    H, H1, H2, H12 = h[:, 0], h[:, 1], h[:, 2], h[:, 3]
    A = H1 - H12
    B = H12
    C = H - H1 - H2 + H12
    D = H2 - H12
    return np.stack([A, B, C, D], axis=-1) * INT_LUMI


def _likelihood(hb: np.ndarray, hs: np.ndarray) -> float:
    """hb/hs: [NBIN, 4] region histograms (A,B,C,D) in float64."""
    from scipy.special import gammaln

    obs_A, obs_B, obs_C, obs_D = hb[:, 0], hb[:, 1], hb[:, 2], hb[:, 3]
    S_A, S_B, S_C, S_D = hs[:, 0], hs[:, 1], hs[:, 2], hs[:, 3]
    mu = 1.0
    # theta = 0, nA/nC/nD = obs_A/obs_C/obs_D
    exp_A = obs_A + mu * S_A
    exp_C = obs_C + mu * S_C
    exp_D = obs_D + mu * S_D
    # (1 + delta) ** theta == 1 at theta = 0
    bkg_SR = obs_A * obs_D / (obs_C + EPS)
    exp_B = bkg_SR + mu * S_B

    def pois(o, e):
        return o * np.log(e + EPS) - e - gammaln(o + 1.0)

    llh = (pois(obs_A, exp_A) + pois(obs_B, exp_B)
           + pois(obs_C, exp_C) + pois(obs_D, exp_D))
    return -float(llh.sum())


_NC_CACHE = None
LAST_RESULTS = None


def kernel(f1_bkg, f2_bkg, mt_bkg, w_bkg, f1_sig, f2_sig, mt_sig, w_sig,
           cut1, cut2, mt_bin_edges):
    global _NC_CACHE, LAST_RESULTS
    from concourse.bass_utils import run_bass_kernel_spmd

    if _NC_CACHE is None:
        _NC_CACHE = _build_program()
    nc = _NC_CACHE

    edges = np.asarray(mt_bin_edges, dtype=np.float64)
    width = float(edges[1] - edges[0])
    e0 = float(edges[0])
    par = np.zeros((P, 16), dtype=np.float32)
    par[:, 0] = -STEEPNESS * float(cut1)
    par[:, 1] = -STEEPNESS * float(cut2)
    par[:, 2] = 1.0 / width
    par[:, 3] = -e0 / width - 0.5
    n_base = Q - QOH_GP - QOH_ACT
    for i in range(QOH_ACT + TAILSHIFT + 2):
        # ACT step slab: sigmoid(SLOPE*(idx - (2m - 0.5))) == [idx >= 2m]
        m = n_base - TAILSHIFT - 2 + i
        par[:, 4 + i] = -STEP_SLOPE * (2.0 * m - 0.5)

    pairs = {
        "f1": (np.asarray(f1_bkg, np.float32), np.asarray(f1_sig, np.float32)),
        "f2": (np.asarray(f2_bkg, np.float32), np.asarray(f2_sig, np.float32)),
        "mt": (np.asarray(mt_bkg, np.float32), np.asarray(mt_sig, np.float32)),
        "w": (np.asarray(w_bkg, np.float32), np.asarray(w_sig, np.float32)),
    }
    chunks = _chunks()

    in_maps = []
    for core in range(NCORES):
        m = {k: _shard_joint(v, core, chunks) for k, v in pairs.items()}
        m["params"] = par
        in_maps.append(m)

    try:
        res = run_bass_kernel_spmd(nc, in_maps, core_ids=list(range(NCORES)))
    except Exception:
        # transient device states (e.g. a wedged exec unit from a prior run)
        # typically clear on retry
        res = run_bass_kernel_spmd(nc, in_maps, core_ids=list(range(NCORES)))
    LAST_RESULTS = res

    total = np.zeros((Q, 2 * NSLAB), dtype=np.float64)
    for rmap in res.results:
        total += rmap["hist_out"].astype(np.float64)

    hb = _regions(_decode(total[:, 0:NSLAB]))
    hs = _regions(_decode(total[:, NSLAB:]))
    out = _likelihood(hb, hs)
    return np.float32(out)


# revision 53
# speedup vs baseline: 1.0002x; 1.0002x over previous
"""Trainium2 Bass kernel for nn_CLsLoss (ABCD soft-region weighted histograms +
profile likelihood).

Strategy (data-parallel over events, 8 cores):
  - Each core gets 1/8 of the 4M bkg events and 1/8 of the 4M sig events,
    reshaped to [128, COLS] (zero-padded weights for the tail). bkg and sig
    chunks are interleaved host-side so each on-device chunk of J columns
    holds J/2 bkg columns followed by J/2 sig columns and every elementwise
    op covers both datasets in one instruction.
  - Per event on-device: sigmoids s1,s2 (ScalarE, bias APs carry the runtime
    cuts); bin index idx = floor((mt-e0)/w) via one ScalarE affine with the
    HW round-to-nearest int16 conversion and a -0.5 bias.
  - Cumulative step slabs instead of one-hots: G_m = [idx >= 2m] for
    m = 0..24; the host recovers the pair histogram as G[m] - G[m+1]
    (G[25] = 0 identically since idx <= 49, so slab 25 is never computed).
    Engine split per the cost model's LP optimum: slab 0 (all ones) is a
    GpSimd memset, ~4 slabs ride ScalarE as saturated sigmoids (same table
    set as the region sigmoids), ~1 on GpSimd, the rest on VectorE is_ge
    (4x mode).
  - Payload: 8 slabs [d | d&mask] where d = (w, w*s1, w*s2, w*s1*s2) in
    bf16. The odd-half product d*(idx&1) is ONE GpSimd tensor_tensor
    bitwise_and over all 4 channels against the broadcast sign-mask
    m16 = -(idx&1) in {0x0000, 0xFFFF} (bit-exact multiply by 0/1, and
    GpSimd's launch overhead amortizes over the 4J-wide fused op, which is
    where GpSimd has comparative advantage per the cost model).
  - Histogram via TensorE: per column t,
      psum[25, 8] += G[128, 25]^T @ SD[128, 8]
    accumulated over all columns per dataset in one PSUM fp32 group.
  - Host: sum per-core [25, 16] partials, difference the steps, map
    (m, odd) -> bin, derive regions A=H1-H12, B=H12, C=H-H1-H2+H12,
    D=H2-H12, scale by INT_LUMI, and evaluate the [50]-bin profile
    likelihood in float64.
"""

import os as _os

import numpy as np

NBIN = 50
N_EVENTS = 4_000_000
NCORES = 8
NPC = N_EVENTS // NCORES          # 500_000 events per core per dataset
P = 128
COLS = int(_os.environ.get("K_COLS", "3920"))   # 128*3920 >= NPC, tail w=0
JMAX = int(_os.environ.get("K_J", "1056"))      # joint chunk width (both ds)
Q = 25                            # pair index m = idx >> 1 (slab 25 == 0)
NCH = 4                           # channels: w, w*s1, w*s2, w*s1*s2
NSLAB = 2 * NCH                   # payload slabs: [d | d*s]
PACK = 4                          # columns per matmul
QOH_GP = int(_os.environ.get("K_QOH_GP", "5"))    # step slabs on GpSimd
QOH_ACT = int(_os.environ.get("K_QOH_ACT", "4"))  # step slabs on ScalarE
QOH_BUFS = int(_os.environ.get("K_QOH_BUFS", "2"))
TAILSHIFT = int(_os.environ.get("K_TAILSHIFT", "2"))  # tail-chunk DVE->ACT steps
HEADSHIFT = int(_os.environ.get("K_HEADSHIFT", "0"))  # head-chunk ACT->DVE steps
# fractional DVE->ACT balance: split one step slab, DVE cols [0:J*num/den),
# ScalarE the rest (the whole-slab quantum overshoots ScalarE's slack)
SPLIT = int(_os.environ.get("K_SPLIT", "1"))          # 1 = enable split slab
SPLIT_NUM = int(_os.environ.get("K_SPLIT_NUM", "8"))
SPLIT_DEN = int(_os.environ.get("K_SPLIT_DEN", "32"))
SPLIT_POOL = int(_os.environ.get("K_SPLIT_POOL", "11"))  # GpSimd's share/den
# ones-slab engine: GpSimd memset (975 ns) vs VectorE is_ge-vs-(-1) (335 ns,
# 4x TSP) - the latter costs 640 ns/chunk less total work
ONES_DVE = int(_os.environ.get("K_ONES_DVE", "0"))
# optional second split slab (same DEN); 0 disables
SPLIT2_NUM = int(_os.environ.get("K_SPLIT2_NUM", "0"))
SPLIT2_POOL = int(_os.environ.get("K_SPLIT2_POOL", "0"))
STEP_SLOPE = 40.0  # sigmoid(+-20) rounds to exactly 1/0 in bf16
INT_LUMI = 117100.0
EPS = 1e-6
STEEPNESS = 20.0


def _chunks():
    """Split per-dataset COLS into chunk widths of at most JMAX//2, each a
    multiple of PACK. The first chunks are small so the pipeline fills
    quickly (less serial head time); the last chunks are small so the final
    matmul burst + drain ceremony hangs off a tiny dependency tail."""
    half = JMAX // 2
    head = [int(x) for x in
            _os.environ.get("K_HEAD", "84,288").split(",") if x]
    tail = [int(x) for x in
            _os.environ.get("K_TAIL", "64").split(",") if x]
    mid = COLS - sum(head) - sum(tail)
    assert mid > 0 and mid % PACK == 0
    widths = list(head)
    while mid > half:
        widths.append(half)
        mid -= half
    if mid:
        widths.append(mid)
    widths += tail
    # chunk = (c0b, wb, c0s, ws): per-dataset start/width. Normally
    # symmetric; with K_ASYM the final chunk is sig-only (its bkg share
    # folds into the penultimate chunk) so the bkg PSUM copy + output-DMA
    # launch ceremony overlaps the last chunk's compute instead of
    # serializing after it.
    wb_list = list(widths)
    ws_list = list(widths)
    if int(_os.environ.get("K_ASYM", "0")) and len(widths) >= 2:
        wb_list[-2] += wb_list[-1]
        wb_list[-1] = 0
    out = []
    c0b = c0s = 0
    for wb, ws in zip(wb_list, ws_list):
        assert wb % PACK == 0 and ws % PACK == 0 and wb + ws > 0
        out.append((c0b, wb, c0s, ws))
        c0b += wb
        c0s += ws
    assert c0b == COLS and c0s == COLS
    return out


def _build_program():
    import concourse.bass as bass
    import concourse.bacc as bacc
    import concourse.mybir as mybir
    import concourse.tile as tile

    dt = mybir.dt
    Alu = mybir.AluOpType
    Act = mybir.ActivationFunctionType

    nc = bacc.Bacc("TRN2", target_bir_lowering=False, debug=False,
                   num_devices=NCORES)

    # Joint inputs: [:, 0:COLS] = bkg, [:, COLS:2*COLS] = sig, chunk-
    # interleaved by the host so one chunk slice is contiguous.
    names = ["f1", "f2", "mt", "w"]
    din = {n: nc.dram_tensor(n, [P, 2 * COLS], dt.float32,
                             kind="ExternalInput")
           for n in names}
    dpar = nc.dram_tensor("params", [P, 16], dt.float32, kind="ExternalInput")
    dout = nc.dram_tensor("hist_out", [Q, 2 * NSLAB], dt.float32,
                          kind="ExternalOutput")

    chunks = _chunks()

    from contextlib import ExitStack
    with tile.TileContext(nc) as tc, ExitStack() as ctx:
        io_pool = ctx.enter_context(tc.tile_pool(
            name="io", bufs=int(_os.environ.get("K_IO_BUFS", "2"))))
        act_pool = ctx.enter_context(tc.tile_pool(
            name="acto", bufs=int(_os.environ.get("K_ACT_BUFS", "2"))))
        sd_pool = ctx.enter_context(tc.tile_pool(
            name="sd", bufs=int(_os.environ.get("K_SD_BUFS", "2"))))
        qoh_pool = ctx.enter_context(tc.tile_pool(name="qoh", bufs=QOH_BUFS))
        const_pool = ctx.enter_context(tc.tile_pool(name="const", bufs=1))
        psum_pool = ctx.enter_context(
            tc.tile_pool(name="psum", bufs=1, space=bass.MemorySpace.PSUM))
        out_pool = ctx.enter_context(tc.tile_pool(name="out", bufs=1))

        par = const_pool.tile([P, 16], dt.float32)
        warm = const_pool.tile([P, 16], dt.bfloat16)
        warm_src = const_pool.tile([P, 16], dt.bfloat16)
        # memset has no input deps, so the sigmoid table load starts at t=0
        # and fully overlaps the params + first-chunk DMAs
        nc.gpsimd.memset(warm_src[:], 0.0)
        # params ride the ACT DGE ring so the first mt chunk is not queued
        # behind them on the SP ring
        nc.scalar.dma_start(par[:], dpar[:])
        bias1 = par[:, 0:1]     # -20*cut1
        bias2 = par[:, 1:2]     # -20*cut2
        invw = par[:, 2:3]      # 1/bin_width
        nege0h = par[:, 3:4]    # -edges[0]/bin_width - 0.5  (floor via rint)
        # touch Sigmoid early so the ACT table set loads during input DMA
        nc.scalar.activation(warm[:], warm_src[:], Act.Sigmoid)

        ps = {ds: psum_pool.tile([Q, NSLAB], dt.float32,
                                 name=f"ps_{ds}", tag=f"ps_{ds}")
              for ds in ("bkg", "sig")}
        started = {"bkg": False, "sig": False}
        n_packs = {"bkg": COLS, "sig": COLS}
        done_packs = {"bkg": 0, "sig": 0}
        drained = set()
        out_sb = out_pool.tile([Q, 2 * NSLAB], dt.float32)

        n_base = Q - QOH_GP - QOH_ACT  # ACT slabs sit below GpSimd's

        for ci, (c0b, wb, c0s, ws) in enumerate(chunks):
            J = wb + ws  # joint width: wb bkg cols then ws sig cols
            off = c0b + c0s  # running dram column offset of this chunk
            w = wb  # bkg/sig boundary within the chunk
            f1 = io_pool.tile([P, J], dt.float32, tag="f1")
            f2 = io_pool.tile([P, J], dt.float32, tag="f2")
            mt = io_pool.tile([P, J], dt.float32, tag="mt")
            wt = io_pool.tile([P, J], dt.float32, tag="w")
            # host interleaves so joint chunk k occupies cols [off, off+J);
            # mt first: it feeds idx16 -> m16 -> all step slabs
            nc.sync.dma_start(mt[:], din["mt"][:, off:off + J])
            nc.sync.dma_start(f1[:], din["f1"][:, off:off + J])
            nc.sync.dma_start(f2[:], din["f2"][:, off:off + J])
            nc.sync.dma_start(wt[:], din["w"][:, off:off + J])

            s12 = act_pool.tile([P, 2 * J], dt.bfloat16, tag="s12")
            s1 = s12[:, 0:J]
            s2 = s12[:, J:2 * J]
            idx16 = act_pool.tile([P, J], dt.int16, tag="idx16")
            # parity mask {0, 1}: min-multiplier for the odd-half payload
            # (payloads are <= w_max ~ 1e-3 < 1, so min(d, parity) == d*parity)
            m16 = act_pool.tile([P, J], dt.int16, tag="m16")
            sd = sd_pool.tile([P, NSLAB * J], dt.bfloat16, tag="sd")

            # idx = floor((mt - e0)/binw) via rint(x - 0.5) on ACT -> int16;
            # first so m16 and the slab engines start as early as possible
            nc.scalar.activation(idx16[:], mt[:], Act.Identity,
                                 bias=nege0h, scale=invw)
            nc.scalar.activation(s1, f1[:], Act.Sigmoid,
                                 bias=bias1, scale=STEEPNESS)
            nc.scalar.activation(s2, f2[:], Act.Sigmoid,
                                 bias=bias2, scale=STEEPNESS)
            # d slab 0 = w in bf16 (ACT copy straight into the payload
            # tile; Identity, not Copy, so no extra act-table load)
            nc.scalar.activation(sd[:, 0:J], wt[:], Act.Identity)

            # small head/tail chunks skip GpSimd for payload+steps: its
            # per-op launch overhead dominates there, and the tail chunks
            # must clear elementwise ASAP so the final matmuls start early
            small = w < int(_os.environ.get("K_SMALL", "0"))

            # m16 = idx & 1 (odd/even split of each bin pair)
            nc.vector.tensor_scalar(m16[:], idx16[:], 1, None,
                                    Alu.bitwise_and)
            # d channels
            # (d1|d2) = w*(s1|s2) in one broadcast TT; d3 = d1*s2 on GpSimd
            # (a mult rides GpSimd's Multiply ucode; the cost model's LP puts
            # d3 + half the odd product there to balance the three engines)
            w_b = sd[:, 0:J].rearrange("p (o t) -> p o t", o=1)
            w_b = w_b.to_broadcast((P, 2, J))
            nc.vector.tensor_tensor(
                sd[:, J:3 * J].rearrange("p (r t) -> p r t", r=2),
                w_b, s12[:].rearrange("p (r t) -> p r t", r=2), Alu.mult)
            nc.vector.tensor_tensor(sd[:, 3 * J:4 * J], sd[:, J:2 * J],
                                    s2, Alu.mult)
            # odd half: d * (idx&1), one broadcast TT. All payload stays on
            # VectorE: GpSimd then depends only on idx16 (pure step slabs),
            # which keeps the cross-engine dependency chains short and the
            # pipeline stall-free
            d_all = sd[:, 0:4 * J].rearrange("p (r t) -> p r t", r=4)
            m_b = m16[:].rearrange("p (o t) -> p o t", o=1)
            nc.vector.tensor_tensor(
                sd[:, 4 * J:8 * J].rearrange("p (r t) -> p r t", r=4),
                m_b.to_broadcast((P, 4, J)), d_all, Alu.mult)

            # cumulative step slabs: G_m[i] = [idx_i >= 2m]. The host
            # recovers pair histograms as G[m] - G[m+1] (G[25] == 0). Slab 0
            # ([idx >= 0] == 1) is a GpSimd memset; QOH_ACT slabs ride
            # ScalarE as saturated sigmoids (same table set as s1/s2);
            # QOH_GP on GpSimd; the rest on VectorE (4x is_ge).
            qoh = qoh_pool.tile([P, Q * J], dt.bfloat16, tag="qoh")
            # tail chunks: shift step slabs off VectorE (it is the engine
            # that finishes last) onto ScalarE, so the final matmul burst
            # starts earlier
            tshift = TAILSHIFT if ci >= len(chunks) - 2 else 0
            # head chunks: ScalarE gates the whole cascade (idx/sigmoids),
            # so move its step slabs to VectorE there
            hshift = HEADSHIFT if ci < 2 else 0
            act_set = set(range(n_base - tshift + hshift, n_base + QOH_ACT))
            # split slab: the one just below the ScalarE range, steady
            # chunks only
            m_split = (n_base - TAILSHIFT - 1
                       if SPLIT and tshift == 0 and hshift == 0 else -1)
            m_split2 = (n_base - TAILSHIFT - 2
                        if SPLIT2_NUM and m_split >= 0 else -1)
            pool_set = (set() if small
                        else set(range(Q - QOH_GP, Q)) - act_set)
            for m in range(Q):
                slab = qoh[:, m * J:(m + 1) * J]
                if m == 0:
                    if ONES_DVE:
                        nc.vector.tensor_scalar(slab, idx16[:], -1.0,
                                                None, Alu.is_ge)
                    else:
                        nc.gpsimd.memset(slab, 1.0)
                elif m in (m_split, m_split2):
                    num, pnum = ((SPLIT_NUM, SPLIT_POOL) if m == m_split
                                 else (SPLIT2_NUM, SPLIT2_POOL))
                    Jd = (J * num // SPLIT_DEN) & ~1
                    Jp = (J * pnum // SPLIT_DEN) & ~1
                    i = m - (n_base - TAILSHIFT - 2)
                    negm = par[:, 4 + i:5 + i]  # -SLOPE*(2m - 0.5)
                    nc.vector.tensor_scalar(slab[:, 0:Jd], idx16[:, 0:Jd],
                                            float(2 * m), None, Alu.is_ge)
                    if Jp:
                        nc.gpsimd.tensor_scalar(
                            slab[:, Jd:Jd + Jp], idx16[:, Jd:Jd + Jp],
                            float(2 * m), None, Alu.is_ge)
                    nc.scalar.activation(slab[:, Jd + Jp:],
                                         idx16[:, Jd + Jp:],
                                         Act.Sigmoid, bias=negm,
                                         scale=STEP_SLOPE)
                elif m in act_set:
                    i = m - (n_base - TAILSHIFT - 2)
                    negm = par[:, 4 + i:5 + i]  # -SLOPE*(2m - 0.5)
                    nc.scalar.activation(slab, idx16[:], Act.Sigmoid,
                                         bias=negm, scale=STEP_SLOPE)
                elif m in pool_set:
                    nc.gpsimd.tensor_scalar(slab, idx16[:], float(2 * m),
                                            None, Alu.is_ge)
                else:
                    nc.vector.tensor_scalar(slab, idx16[:], float(2 * m),
                                            None, Alu.is_ge)

            # per-column matmuls (hw: the weights AP allows only one free
            # dim, so columns cannot be packed into a wider stationary)
            qoh_r = qoh[:].rearrange("p (m t) -> p t m", t=J)
            sd_r = sd[:].rearrange("p (j t) -> p t j", t=J)
            for t0 in range(J):
                ds = "bkg" if t0 < w else "sig"
                first = not started[ds]
                started[ds] = True
                done_packs[ds] += 1
                last = done_packs[ds] == n_packs[ds]
                nc.tensor.matmul(
                    ps[ds][:], qoh_r[:, t0, :], sd_r[:, t0, :],
                    start=first, stop=last, skip_group_check=True)

            # with the asymmetric tail, bkg completes one chunk early: its
            # PSUM copy + output-DMA launch ceremony (~1.5us of sem/DGE
            # fixed cost) overlaps the final chunk's compute instead of
            # serializing after it. ScalarE does the early copy - it is
            # idle at the tail while VectorE is the critical engine.
            if (done_packs["bkg"] == n_packs["bkg"] and "bkg" not in drained
                    and ci < len(chunks) - 1):
                drained.add("bkg")
                nc.scalar.activation(out_sb[:, 0:NSLAB], ps["bkg"][:],
                                     Act.Identity)
                nc.sync.dma_start(dout[:, 0:NSLAB], out_sb[:, 0:NSLAB])

        out_ring = (nc.scalar if _os.environ.get("K_OUTRING", "sync")
                    == "scalar" else nc.sync)
        if "bkg" in drained:
            nc.vector.tensor_copy(out_sb[:, NSLAB:], ps["sig"][:])
            out_ring.dma_start(dout[:, NSLAB:], out_sb[:, NSLAB:])
        else:
            nc.vector.tensor_copy(out_sb[:, 0:NSLAB], ps["bkg"][:])
            nc.vector.tensor_copy(out_sb[:, NSLAB:], ps["sig"][:])
            out_ring.dma_start(dout[:], out_sb[:])

    nc.compile()
    return nc


def _shard_joint(arr: np.ndarray, core: int, chunks) -> np.ndarray:
    """arr: (bkg_full, sig_full) pair -> [P, 2*COLS] chunk-interleaved."""
    bkg_full, sig_full = arr
    out = np.zeros((P, 2 * COLS), dtype=np.float32)
    halves = []
    for full in (bkg_full, sig_full):
        sl = full[core * NPC:(core + 1) * NPC]
        h = np.zeros(P * COLS, dtype=np.float32)
        h[:NPC] = sl
        halves.append(h.reshape(P, COLS))
    b, s = halves
    for c0b, wb, c0s, ws in chunks:
        off = c0b + c0s
        out[:, off:off + wb] = b[:, c0b:c0b + wb]
        out[:, off + wb:off + wb + ws] = s[:, c0s:c0s + ws]
    return out


def _decode(block: np.ndarray) -> np.ndarray:
    """[Q, NSLAB] psum block of cumulative steps -> [NBIN, NCH] histogram.

    Row m holds G[m] = sum over events with idx >= 2m; pair m is
    G[m] - G[m+1] (G[25] = 0 since idx <= 49)."""
    pair = block.astype(np.float64).copy()
    pair[:-1] -= block[1:]
    h_all = pair[:, 0:NCH]
    h_odd = pair[:, NCH:2 * NCH]
    h_even = h_all - h_odd
    bins = np.empty((2 * Q, NCH))
    bins[0::2] = h_even
    bins[1::2] = h_odd
    return bins[:NBIN]


def _regions(h: np.ndarray) -> np.ndarray:
    """[NBIN, 4] channel hist (H, H1, H2, H12) -> regions (A,B,C,D)*lumi."""
    H, H1, H2, H12 = h[:, 0], h[:, 1], h[:, 2], h[:, 3]
    A = H1 - H12
    B = H12
    C = H - H1 - H2 + H12
    D = H2 - H12
    return np.stack([A, B, C, D], axis=-1) * INT_LUMI


def _likelihood(hb: np.ndarray, hs: np.ndarray) -> float:
    """hb/hs: [NBIN, 4] region histograms (A,B,C,D) in float64."""
    from scipy.special import gammaln

    obs_A, obs_B, obs_C, obs_D = hb[:, 0], hb[:, 1], hb[:, 2], hb[:, 3]
    S_A, S_B, S_C, S_D = hs[:, 0], hs[:, 1], hs[:, 2], hs[:, 3]
    mu = 1.0
    # theta = 0, nA/nC/nD = obs_A/obs_C/obs_D
    exp_A = obs_A + mu * S_A
    exp_C = obs_C + mu * S_C
    exp_D = obs_D + mu * S_D
    # (1 + delta) ** theta == 1 at theta = 0
    bkg_SR = obs_A * obs_D / (obs_C + EPS)
    exp_B = bkg_SR + mu * S_B

    def pois(o, e):
        return o * np.log(e + EPS) - e - gammaln(o + 1.0)

    llh = (pois(obs_A, exp_A) + pois(obs_B, exp_B)
           + pois(obs_C, exp_C) + pois(obs_D, exp_D))
    return -float(llh.sum())


_NC_CACHE = None
LAST_RESULTS = None


def kernel(f1_bkg, f2_bkg, mt_bkg, w_bkg, f1_sig, f2_sig, mt_sig, w_sig,
           cut1, cut2, mt_bin_edges):
    global _NC_CACHE, LAST_RESULTS
    from concourse.bass_utils import run_bass_kernel_spmd

    if _NC_CACHE is None:
        _NC_CACHE = _build_program()
    nc = _NC_CACHE

    edges = np.asarray(mt_bin_edges, dtype=np.float64)
    width = float(edges[1] - edges[0])
    e0 = float(edges[0])
    par = np.zeros((P, 16), dtype=np.float32)
    par[:, 0] = -STEEPNESS * float(cut1)
    par[:, 1] = -STEEPNESS * float(cut2)
    par[:, 2] = 1.0 / width
    par[:, 3] = -e0 / width - 0.5
    n_base = Q - QOH_GP - QOH_ACT
    for i in range(QOH_ACT + TAILSHIFT + 2):
        # ACT step slab: sigmoid(SLOPE*(idx - (2m - 0.5))) == [idx >= 2m]
        m = n_base - TAILSHIFT - 2 + i
        par[:, 4 + i] = -STEP_SLOPE * (2.0 * m - 0.5)

    pairs = {
        "f1": (np.asarray(f1_bkg, np.float32), np.asarray(f1_sig, np.float32)),
        "f2": (np.asarray(f2_bkg, np.float32), np.asarray(f2_sig, np.float32)),
        "mt": (np.asarray(mt_bkg, np.float32), np.asarray(mt_sig, np.float32)),
        "w": (np.asarray(w_bkg, np.float32), np.asarray(w_sig, np.float32)),
    }
    chunks = _chunks()

    in_maps = []
    for core in range(NCORES):
        m = {k: _shard_joint(v, core, chunks) for k, v in pairs.items()}
        m["params"] = par
        in_maps.append(m)

    try:
        res = run_bass_kernel_spmd(nc, in_maps, core_ids=list(range(NCORES)))
    except Exception:
        # transient device states (e.g. a wedged exec unit from a prior run)
        # typically clear on retry
        res = run_bass_kernel_spmd(nc, in_maps, core_ids=list(range(NCORES)))
    LAST_RESULTS = res

    total = np.zeros((Q, 2 * NSLAB), dtype=np.float64)
    for rmap in res.results:
        total += rmap["hist_out"].astype(np.float64)

    hb = _regions(_decode(total[:, 0:NSLAB]))
    hs = _regions(_decode(total[:, NSLAB:]))
    out = _likelihood(hb, hs)
    return np.float32(out)


# revision 56
# speedup vs baseline: 1.0006x; 1.0004x over previous
"""Trainium2 Bass kernel for nn_CLsLoss (ABCD soft-region weighted histograms +
profile likelihood).

Strategy (data-parallel over events, 8 cores):
  - Each core gets 1/8 of the 4M bkg events and 1/8 of the 4M sig events,
    reshaped to [128, COLS] (zero-padded weights for the tail). bkg and sig
    chunks are interleaved host-side so each on-device chunk of J columns
    holds J/2 bkg columns followed by J/2 sig columns and every elementwise
    op covers both datasets in one instruction.
  - Per event on-device: sigmoids s1,s2 (ScalarE, bias APs carry the runtime
    cuts); bin index idx = floor((mt-e0)/w) via one ScalarE affine with the
    HW round-to-nearest int16 conversion and a -0.5 bias.
  - Cumulative step slabs instead of one-hots: G_m = [idx >= 2m] for
    m = 0..24; the host recovers the pair histogram as G[m] - G[m+1]
    (G[25] = 0 identically since idx <= 49, so slab 25 is never computed).
    Engine split per the cost model's LP optimum: slab 0 (all ones) is a
    GpSimd memset, ~4 slabs ride ScalarE as saturated sigmoids (same table
    set as the region sigmoids), ~1 on GpSimd, the rest on VectorE is_ge
    (4x mode).
  - Payload: 8 slabs [d | d&mask] where d = (w, w*s1, w*s2, w*s1*s2) in
    bf16. The odd-half product d*(idx&1) is ONE GpSimd tensor_tensor
    bitwise_and over all 4 channels against the broadcast sign-mask
    m16 = -(idx&1) in {0x0000, 0xFFFF} (bit-exact multiply by 0/1, and
    GpSimd's launch overhead amortizes over the 4J-wide fused op, which is
    where GpSimd has comparative advantage per the cost model).
  - Histogram via TensorE: per column t,
      psum[25, 8] += G[128, 25]^T @ SD[128, 8]
    accumulated over all columns per dataset in one PSUM fp32 group.
  - Host: sum per-core [25, 16] partials, difference the steps, map
    (m, odd) -> bin, derive regions A=H1-H12, B=H12, C=H-H1-H2+H12,
    D=H2-H12, scale by INT_LUMI, and evaluate the [50]-bin profile
    likelihood in float64.
"""

import os as _os

import numpy as np

NBIN = 50
N_EVENTS = 4_000_000
NCORES = 8
NPC = N_EVENTS // NCORES          # 500_000 events per core per dataset
P = 128
COLS = int(_os.environ.get("K_COLS", "3920"))   # 128*3920 >= NPC, tail w=0
JMAX = int(_os.environ.get("K_J", "1056"))      # joint chunk width (both ds)
Q = 25                            # pair index m = idx >> 1 (slab 25 == 0)
NCH = 4                           # channels: w, w*s1, w*s2, w*s1*s2
NSLAB = 2 * NCH                   # payload slabs: [d | d*s]
PACK = 4                          # columns per matmul
QOH_GP = int(_os.environ.get("K_QOH_GP", "5"))    # step slabs on GpSimd
QOH_ACT = int(_os.environ.get("K_QOH_ACT", "4"))  # step slabs on ScalarE
QOH_BUFS = int(_os.environ.get("K_QOH_BUFS", "2"))
TAILSHIFT = int(_os.environ.get("K_TAILSHIFT", "2"))  # tail-chunk DVE->ACT steps
HEADSHIFT = int(_os.environ.get("K_HEADSHIFT", "0"))  # head-chunk ACT->DVE steps
# fractional DVE->ACT balance: split one step slab, DVE cols [0:J*num/den),
# ScalarE the rest (the whole-slab quantum overshoots ScalarE's slack)
SPLIT = int(_os.environ.get("K_SPLIT", "1"))          # 1 = enable split slab
SPLIT_NUM = int(_os.environ.get("K_SPLIT_NUM", "8"))
SPLIT_DEN = int(_os.environ.get("K_SPLIT_DEN", "32"))
SPLIT_POOL = int(_os.environ.get("K_SPLIT_POOL", "11"))  # GpSimd's share/den
# ones-slab engine: GpSimd memset (975 ns) vs VectorE is_ge-vs-(-1) (335 ns,
# 4x TSP) - the latter costs 640 ns/chunk less total work
ONES_DVE = int(_os.environ.get("K_ONES_DVE", "0"))
# totals row via a second per-column matmul against a constant ones
# stationary (PE has slack); kills the per-chunk GpSimd ones-memset
TOTMM = int(_os.environ.get("K_TOTMM", "0"))
# optional second split slab (same DEN); 0 disables
SPLIT2_NUM = int(_os.environ.get("K_SPLIT2_NUM", "0"))
SPLIT2_POOL = int(_os.environ.get("K_SPLIT2_POOL", "0"))
STEP_SLOPE = 40.0  # sigmoid(+-20) rounds to exactly 1/0 in bf16
INT_LUMI = 117100.0
EPS = 1e-6
STEEPNESS = 20.0


def _chunks():
    """Split per-dataset COLS into chunk widths of at most JMAX//2, each a
    multiple of PACK. The first chunks are small so the pipeline fills
    quickly (less serial head time); the last chunks are small so the final
    matmul burst + drain ceremony hangs off a tiny dependency tail."""
    half = JMAX // 2
    head = [int(x) for x in
            _os.environ.get("K_HEAD", "84,288").split(",") if x]
    tail = [int(x) for x in
            _os.environ.get("K_TAIL", "64").split(",") if x]
    mid = COLS - sum(head) - sum(tail)
    assert mid > 0 and mid % PACK == 0
    widths = list(head)
    while mid > half:
        widths.append(half)
        mid -= half
    if mid:
        widths.append(mid)
    widths += tail
    # chunk = (c0b, wb, c0s, ws): per-dataset start/width. Normally
    # symmetric; with K_ASYM the final chunk is sig-only (its bkg share
    # folds into the penultimate chunk) so the bkg PSUM copy + output-DMA
    # launch ceremony overlaps the last chunk's compute instead of
    # serializing after it.
    wb_list = list(widths)
    ws_list = list(widths)
    if int(_os.environ.get("K_ASYM", "0")) and len(widths) >= 2:
        wb_list[-2] += wb_list[-1]
        wb_list[-1] = 0
    out = []
    c0b = c0s = 0
    for wb, ws in zip(wb_list, ws_list):
        assert wb % PACK == 0 and ws % PACK == 0 and wb + ws > 0
        out.append((c0b, wb, c0s, ws))
        c0b += wb
        c0s += ws
    assert c0b == COLS and c0s == COLS
    return out


def _build_program():
    import concourse.bass as bass
    import concourse.bacc as bacc
    import concourse.mybir as mybir
    import concourse.tile as tile

    dt = mybir.dt
    Alu = mybir.AluOpType
    Act = mybir.ActivationFunctionType

    nc = bacc.Bacc("TRN2", target_bir_lowering=False, debug=False,
                   num_devices=NCORES)

    # Joint inputs: [:, 0:COLS] = bkg, [:, COLS:2*COLS] = sig, chunk-
    # interleaved by the host so one chunk slice is contiguous.
    names = ["f1", "f2", "mt", "w"]
    din = {n: nc.dram_tensor(n, [P, 2 * COLS], dt.float32,
                             kind="ExternalInput")
           for n in names}
    dpar = nc.dram_tensor("params", [P, 16], dt.float32, kind="ExternalInput")
    dout = nc.dram_tensor("hist_out", [Q, 2 * NSLAB], dt.float32,
                          kind="ExternalOutput")

    chunks = _chunks()

    from contextlib import ExitStack
    with tile.TileContext(nc) as tc, ExitStack() as ctx:
        io_pool = ctx.enter_context(tc.tile_pool(
            name="io", bufs=int(_os.environ.get("K_IO_BUFS", "2"))))
        act_pool = ctx.enter_context(tc.tile_pool(
            name="acto", bufs=int(_os.environ.get("K_ACT_BUFS", "2"))))
        sd_pool = ctx.enter_context(tc.tile_pool(
            name="sd", bufs=int(_os.environ.get("K_SD_BUFS", "2"))))
        qoh_pool = ctx.enter_context(tc.tile_pool(name="qoh", bufs=QOH_BUFS))
        const_pool = ctx.enter_context(tc.tile_pool(name="const", bufs=1))
        psum_pool = ctx.enter_context(
            tc.tile_pool(name="psum", bufs=1, space=bass.MemorySpace.PSUM))
        out_pool = ctx.enter_context(tc.tile_pool(name="out", bufs=1))

        par = const_pool.tile([P, 16], dt.float32)
        warm = const_pool.tile([P, 16], dt.bfloat16)
        warm_src = const_pool.tile([P, 16], dt.bfloat16)
        # memset has no input deps, so the sigmoid table load starts at t=0
        # and fully overlaps the params + first-chunk DMAs
        nc.gpsimd.memset(warm_src[:], 0.0)
        # params ride the ACT DGE ring so the first mt chunk is not queued
        # behind them on the SP ring
        nc.scalar.dma_start(par[:], dpar[:])
        bias1 = par[:, 0:1]     # -20*cut1
        bias2 = par[:, 1:2]     # -20*cut2
        invw = par[:, 2:3]      # 1/bin_width
        nege0h = par[:, 3:4]    # -edges[0]/bin_width - 0.5  (floor via rint)
        # touch Sigmoid early so the ACT table set loads during input DMA
        nc.scalar.activation(warm[:], warm_src[:], Act.Sigmoid)
        if TOTMM:
            ones1 = const_pool.tile([P, 1], dt.bfloat16)
            nc.gpsimd.memset(ones1[:], 1.0)

        nq = Q - 1 if TOTMM else Q  # stationary slab rows per matmul
        ps = {ds: psum_pool.tile([nq, NSLAB], dt.float32,
                                 name=f"ps_{ds}", tag=f"ps_{ds}")
              for ds in ("bkg", "sig")}
        pt = ({ds: psum_pool.tile([1, NSLAB], dt.float32,
                                  name=f"pt_{ds}", tag=f"pt_{ds}")
               for ds in ("bkg", "sig")} if TOTMM else None)
        started = {"bkg": False, "sig": False}
        n_packs = {"bkg": COLS, "sig": COLS}
        done_packs = {"bkg": 0, "sig": 0}
        drained = set()
        out_sb = out_pool.tile([Q, 2 * NSLAB], dt.float32)

        n_base = Q - QOH_GP - QOH_ACT  # ACT slabs sit below GpSimd's

        for ci, (c0b, wb, c0s, ws) in enumerate(chunks):
            J = wb + ws  # joint width: wb bkg cols then ws sig cols
            off = c0b + c0s  # running dram column offset of this chunk
            w = wb  # bkg/sig boundary within the chunk
            f1 = io_pool.tile([P, J], dt.float32, tag="f1")
            f2 = io_pool.tile([P, J], dt.float32, tag="f2")
            mt = io_pool.tile([P, J], dt.float32, tag="mt")
            wt = io_pool.tile([P, J], dt.float32, tag="w")
            # host interleaves so joint chunk k occupies cols [off, off+J);
            # mt first: it feeds idx16 -> m16 -> all step slabs
            nc.sync.dma_start(mt[:], din["mt"][:, off:off + J])
            nc.sync.dma_start(f1[:], din["f1"][:, off:off + J])
            nc.sync.dma_start(f2[:], din["f2"][:, off:off + J])
            nc.sync.dma_start(wt[:], din["w"][:, off:off + J])

            s12 = act_pool.tile([P, 2 * J], dt.bfloat16, tag="s12")
            s1 = s12[:, 0:J]
            s2 = s12[:, J:2 * J]
            idx16 = act_pool.tile([P, J], dt.int16, tag="idx16")
            # parity mask {0, 1}: min-multiplier for the odd-half payload
            # (payloads are <= w_max ~ 1e-3 < 1, so min(d, parity) == d*parity)
            m16 = act_pool.tile([P, J], dt.int16, tag="m16")
            sd = sd_pool.tile([P, NSLAB * J], dt.bfloat16, tag="sd")

            # idx = floor((mt - e0)/binw) via rint(x - 0.5) on ACT -> int16;
            # first so m16 and the slab engines start as early as possible
            nc.scalar.activation(idx16[:], mt[:], Act.Identity,
                                 bias=nege0h, scale=invw)
            nc.scalar.activation(s1, f1[:], Act.Sigmoid,
                                 bias=bias1, scale=STEEPNESS)
            nc.scalar.activation(s2, f2[:], Act.Sigmoid,
                                 bias=bias2, scale=STEEPNESS)
            # d slab 0 = w in bf16 (ACT copy straight into the payload
            # tile; Identity, not Copy, so no extra act-table load)
            nc.scalar.activation(sd[:, 0:J], wt[:], Act.Identity)

            # small head/tail chunks skip GpSimd for payload+steps: its
            # per-op launch overhead dominates there, and the tail chunks
            # must clear elementwise ASAP so the final matmuls start early
            small = w < int(_os.environ.get("K_SMALL", "0"))

            # m16 = idx & 1 (odd/even split of each bin pair)
            nc.vector.tensor_scalar(m16[:], idx16[:], 1, None,
                                    Alu.bitwise_and)
            # d channels
            # (d1|d2) = w*(s1|s2) in one broadcast TT; d3 = d1*s2 on GpSimd
            # (a mult rides GpSimd's Multiply ucode; the cost model's LP puts
            # d3 + half the odd product there to balance the three engines)
            w_b = sd[:, 0:J].rearrange("p (o t) -> p o t", o=1)
            w_b = w_b.to_broadcast((P, 2, J))
            nc.vector.tensor_tensor(
                sd[:, J:3 * J].rearrange("p (r t) -> p r t", r=2),
                w_b, s12[:].rearrange("p (r t) -> p r t", r=2), Alu.mult)
            nc.vector.tensor_tensor(sd[:, 3 * J:4 * J], sd[:, J:2 * J],
                                    s2, Alu.mult)
            # odd half: d * (idx&1), one broadcast TT. All payload stays on
            # VectorE: GpSimd then depends only on idx16 (pure step slabs),
            # which keeps the cross-engine dependency chains short and the
            # pipeline stall-free
            d_all = sd[:, 0:4 * J].rearrange("p (r t) -> p r t", r=4)
            m_b = m16[:].rearrange("p (o t) -> p o t", o=1)
            nc.vector.tensor_tensor(
                sd[:, 4 * J:8 * J].rearrange("p (r t) -> p r t", r=4),
                m_b.to_broadcast((P, 4, J)), d_all, Alu.mult)

            # cumulative step slabs: G_m[i] = [idx_i >= 2m]. The host
            # recovers pair histograms as G[m] - G[m+1] (G[25] == 0). Slab 0
            # ([idx >= 0] == 1) is a GpSimd memset; QOH_ACT slabs ride
            # ScalarE as saturated sigmoids (same table set as s1/s2);
            # QOH_GP on GpSimd; the rest on VectorE (4x is_ge).
            qoh = qoh_pool.tile([P, nq * J], dt.bfloat16, tag="qoh")
            # tail chunks: shift step slabs off VectorE (it is the engine
            # that finishes last) onto ScalarE, so the final matmul burst
            # starts earlier
            tshift = TAILSHIFT if ci >= len(chunks) - 2 else 0
            # head chunks: ScalarE gates the whole cascade (idx/sigmoids),
            # so move its step slabs to VectorE there
            hshift = HEADSHIFT if ci < 2 else 0
            act_set = set(range(n_base - tshift + hshift, n_base + QOH_ACT))
            # split slab: the one just below the ScalarE range, steady
            # chunks only
            m_split = (n_base - TAILSHIFT - 1
                       if SPLIT and tshift == 0 and hshift == 0 else -1)
            m_split2 = (n_base - TAILSHIFT - 2
                        if SPLIT2_NUM and m_split >= 0 else -1)
            if small:
                pool_set = set()
            elif int(_os.environ.get("K_GPLOW", "1")):
                pool_set = set(range(1, 1 + QOH_GP)) - act_set
            else:
                pool_set = set(range(Q - QOH_GP, Q)) - act_set
            for m in range(Q):
                if TOTMM and m == 0:
                    continue  # totals row comes from the ones-matmul
                ms = m - 1 if TOTMM else m
                slab = qoh[:, ms * J:(ms + 1) * J]
                if m == 0:
                    if ONES_DVE:
                        nc.vector.tensor_scalar(slab, idx16[:], -1.0,
                                                None, Alu.is_ge)
                    else:
                        nc.gpsimd.memset(slab, 1.0)
                elif m in (m_split, m_split2):
                    num, pnum = ((SPLIT_NUM, SPLIT_POOL) if m == m_split
                                 else (SPLIT2_NUM, SPLIT2_POOL))
                    Jd = (J * num // SPLIT_DEN) & ~1
                    Jp = (J * pnum // SPLIT_DEN) & ~1
                    i = m - (n_base - TAILSHIFT - 2)
                    negm = par[:, 4 + i:5 + i]  # -SLOPE*(2m - 0.5)
                    nc.vector.tensor_scalar(slab[:, 0:Jd], idx16[:, 0:Jd],
                                            float(2 * m), None, Alu.is_ge)
                    if Jp:
                        nc.gpsimd.tensor_scalar(
                            slab[:, Jd:Jd + Jp], idx16[:, Jd:Jd + Jp],
                            float(2 * m), None, Alu.is_ge)
                    nc.scalar.activation(slab[:, Jd + Jp:],
                                         idx16[:, Jd + Jp:],
                                         Act.Sigmoid, bias=negm,
                                         scale=STEP_SLOPE)
                elif m in act_set:
                    i = m - (n_base - TAILSHIFT - 2)
                    negm = par[:, 4 + i:5 + i]  # -SLOPE*(2m - 0.5)
                    nc.scalar.activation(slab, idx16[:], Act.Sigmoid,
                                         bias=negm, scale=STEP_SLOPE)
                elif m in pool_set:
                    nc.gpsimd.tensor_scalar(slab, idx16[:], float(2 * m),
                                            None, Alu.is_ge)
                else:
                    nc.vector.tensor_scalar(slab, idx16[:], float(2 * m),
                                            None, Alu.is_ge)

            # per-column matmuls (hw: the weights AP allows only one free
            # dim, so columns cannot be packed into a wider stationary)
            qoh_r = qoh[:].rearrange("p (m t) -> p t m", t=J)
            sd_r = sd[:].rearrange("p (j t) -> p t j", t=J)
            for t0 in range(J):
                ds = "bkg" if t0 < w else "sig"
                first = not started[ds]
                started[ds] = True
                done_packs[ds] += 1
                last = done_packs[ds] == n_packs[ds]
                nc.tensor.matmul(
                    ps[ds][:], qoh_r[:, t0, :], sd_r[:, t0, :],
                    start=first, stop=last, skip_group_check=True)
                if TOTMM:
                    nc.tensor.matmul(
                        pt[ds][:], ones1[:], sd_r[:, t0, :],
                        start=first, stop=last, skip_group_check=True)

            # with the asymmetric tail, bkg completes one chunk early: its
            # PSUM copy + output-DMA launch ceremony (~1.5us of sem/DGE
            # fixed cost) overlaps the final chunk's compute instead of
            # serializing after it. ScalarE does the early copy - it is
            # idle at the tail while VectorE is the critical engine.
            if (done_packs["bkg"] == n_packs["bkg"] and "bkg" not in drained
                    and ci < len(chunks) - 1):
                drained.add("bkg")
                nc.scalar.activation(out_sb[:, 0:NSLAB], ps["bkg"][:],
                                     Act.Identity)
                nc.sync.dma_start(dout[:, 0:NSLAB], out_sb[:, 0:NSLAB])

        out_ring = (nc.scalar if _os.environ.get("K_OUTRING", "sync")
                    == "scalar" else nc.sync)
        if TOTMM:
            for di, ds in enumerate(("bkg", "sig")):
                sl = slice(di * NSLAB, (di + 1) * NSLAB)
                nc.vector.tensor_copy(out_sb[0:1, sl], pt[ds][:])
                nc.vector.tensor_copy(out_sb[1:Q, sl], ps[ds][:])
            out_ring.dma_start(dout[:], out_sb[:])
        elif "bkg" in drained:
            nc.vector.tensor_copy(out_sb[:, NSLAB:], ps["sig"][:])
            out_ring.dma_start(dout[:, NSLAB:], out_sb[:, NSLAB:])
        else:
            nc.vector.tensor_copy(out_sb[:, 0:NSLAB], ps["bkg"][:])
            nc.vector.tensor_copy(out_sb[:, NSLAB:], ps["sig"][:])
            out_ring.dma_start(dout[:], out_sb[:])

    nc.compile()
    return nc


def _shard_joint(arr: np.ndarray, core: int, chunks) -> np.ndarray:
    """arr: (bkg_full, sig_full) pair -> [P, 2*COLS] chunk-interleaved."""
    bkg_full, sig_full = arr
    out = np.zeros((P, 2 * COLS), dtype=np.float32)
    halves = []
    for full in (bkg_full, sig_full):
        sl = full[core * NPC:(core + 1) * NPC]
        h = np.zeros(P * COLS, dtype=np.float32)
        h[:NPC] = sl
        halves.append(h.reshape(P, COLS))
    b, s = halves
    for c0b, wb, c0s, ws in chunks:
        off = c0b + c0s
        out[:, off:off + wb] = b[:, c0b:c0b + wb]
        out[:, off + wb:off + wb + ws] = s[:, c0s:c0s + ws]
    return out


def _decode(block: np.ndarray) -> np.ndarray:
    """[Q, NSLAB] psum block of cumulative steps -> [NBIN, NCH] histogram.

    Row m holds G[m] = sum over events with idx >= 2m; pair m is
    G[m] - G[m+1] (G[25] = 0 since idx <= 49)."""
    pair = block.astype(np.float64).copy()
    pair[:-1] -= block[1:]
    h_all = pair[:, 0:NCH]
    h_odd = pair[:, NCH:2 * NCH]
    h_even = h_all - h_odd
    bins = np.empty((2 * Q, NCH))
    bins[0::2] = h_even
    bins[1::2] = h_odd
    return bins[:NBIN]


def _regions(h: np.ndarray) -> np.ndarray:
    """[NBIN, 4] channel hist (H, H1, H2, H12) -> regions (A,B,C,D)*lumi."""
    H, H1, H2, H12 = h[:, 0], h[:, 1], h[:, 2], h[:, 3]
    A = H1 - H12
    B = H12
    C = H - H1 - H2 + H12
    D = H2 - H12
    return np.stack([A, B, C, D], axis=-1) * INT_LUMI


def _likelihood(hb: np.ndarray, hs: np.ndarray) -> float:
    """hb/hs: [NBIN, 4] region histograms (A,B,C,D) in float64."""
    from scipy.special import gammaln

    obs_A, obs_B, obs_C, obs_D = hb[:, 0], hb[:, 1], hb[:, 2], hb[:, 3]
    S_A, S_B, S_C, S_D = hs[:, 0], hs[:, 1], hs[:, 2], hs[:, 3]
    mu = 1.0
    # theta = 0, nA/nC/nD = obs_A/obs_C/obs_D
    exp_A = obs_A + mu * S_A
    exp_C = obs_C + mu * S_C
    exp_D = obs_D + mu * S_D
    # (1 + delta) ** theta == 1 at theta = 0
    bkg_SR = obs_A * obs_D / (obs_C + EPS)
    exp_B = bkg_SR + mu * S_B

    def pois(o, e):
        return o * np.log(e + EPS) - e - gammaln(o + 1.0)

    llh = (pois(obs_A, exp_A) + pois(obs_B, exp_B)
           + pois(obs_C, exp_C) + pois(obs_D, exp_D))
    return -float(llh.sum())


_NC_CACHE = None
LAST_RESULTS = None


def kernel(f1_bkg, f2_bkg, mt_bkg, w_bkg, f1_sig, f2_sig, mt_sig, w_sig,
           cut1, cut2, mt_bin_edges):
    global _NC_CACHE, LAST_RESULTS
    from concourse.bass_utils import run_bass_kernel_spmd

    if _NC_CACHE is None:
        _NC_CACHE = _build_program()
    nc = _NC_CACHE

    edges = np.asarray(mt_bin_edges, dtype=np.float64)
    width = float(edges[1] - edges[0])
    e0 = float(edges[0])
    par = np.zeros((P, 16), dtype=np.float32)
    par[:, 0] = -STEEPNESS * float(cut1)
    par[:, 1] = -STEEPNESS * float(cut2)
    par[:, 2] = 1.0 / width
    par[:, 3] = -e0 / width - 0.5
    n_base = Q - QOH_GP - QOH_ACT
    for i in range(QOH_ACT + TAILSHIFT + 2):
        # ACT step slab: sigmoid(SLOPE*(idx - (2m - 0.5))) == [idx >= 2m]
        m = n_base - TAILSHIFT - 2 + i
        par[:, 4 + i] = -STEP_SLOPE * (2.0 * m - 0.5)

    pairs = {
        "f1": (np.asarray(f1_bkg, np.float32), np.asarray(f1_sig, np.float32)),
        "f2": (np.asarray(f2_bkg, np.float32), np.asarray(f2_sig, np.float32)),
        "mt": (np.asarray(mt_bkg, np.float32), np.asarray(mt_sig, np.float32)),
        "w": (np.asarray(w_bkg, np.float32), np.asarray(w_sig, np.float32)),
    }
    chunks = _chunks()

    in_maps = []
    for core in range(NCORES):
        m = {k: _shard_joint(v, core, chunks) for k, v in pairs.items()}
        m["params"] = par
        in_maps.append(m)

    try:
        res = run_bass_kernel_spmd(nc, in_maps, core_ids=list(range(NCORES)))
    except Exception:
        # transient device states (e.g. a wedged exec unit from a prior run)
        # typically clear on retry
        res = run_bass_kernel_spmd(nc, in_maps, core_ids=list(range(NCORES)))
    LAST_RESULTS = res

    total = np.zeros((Q, 2 * NSLAB), dtype=np.float64)
    for rmap in res.results:
        total += rmap["hist_out"].astype(np.float64)

    hb = _regions(_decode(total[:, 0:NSLAB]))
    hs = _regions(_decode(total[:, NSLAB:]))
    out = _likelihood(hb, hs)
    return np.float32(out)
